# revision 15
# baseline (speedup 1.0000x reference)
"""Trainium2 Bass kernel for CausalMessagePassingLayer (GCN message passing).

Contract: kernel(**inputs) takes FULL unsharded inputs (numpy), returns the
FULL output. Internally shards batch B=16 across 8 NeuronCores (2 samples per
core), builds one SPMD Bass/Tile program, and runs it via
run_bass_kernel_spmd.

Math (per sample), with x = t_emb[t2e], A-hat = binary adjacency incl.
self-loops, dinv = 1/sqrt(deg):
    y0   = dinv * x                       (row scale)
    agg  = A_hat @ y0                     (message aggregation, binary one-hot
                                           matmuls over col-sorted messages)
    z    = dinv * agg                     (row scale)
    gnn  = z @ W.T + b
    causal[j] = gnn[j-1], causal[0] = 0   (folded into scatter indices)
    out  = t_emb;  out[e2t[j]] += causal[j]
"""
import os
import numpy as np
from contextlib import ExitStack

import concourse.bacc as bacc
import concourse.mybir as mybir
from concourse import tile, library_config
from concourse.bass_utils import run_bass_kernel_spmd

F32 = mybir.dt.float32
BF16 = mybir.dt.bfloat16
I16 = mybir.dt.int16
BF16_NP = mybir.dt.np(BF16)

B, S, D, E, M = 16, 8192, 256, 4096, 32768
NCORES, SPC = 8, 2          # cores, samples per core
NCT = E // 128              # 32 c-tiles per sample
KH = D // 128               # 2 contraction halves


def _wrap_idx(ix):
    """Wrapped SWDGE index layout: [128, n//16]; idx i at [i%16, i//16],
    replicated across the 8 Q7 cores (16-partition groups)."""
    n = ix.shape[0]
    w = ix.reshape(n // 16, 16).T.astype(np.int16)
    return np.tile(w, (8, 1))


def _prep_sample(row, col, t2e, e2t, bpc):
    """Host-side index preprocessing for one sample."""
    deg = 1.0 + np.bincount(col, minlength=E)
    dinv = (1.0 / np.sqrt(deg)).astype(np.float32)

    rows_all = np.concatenate([row, np.arange(E)])   # self-loops as messages
    cols_all = np.concatenate([col, np.arange(E)])
    order = np.argsort(cols_all, kind="stable")
    r_s, c_s = rows_all[order], cols_all[order]

    cnt = np.bincount(c_s >> 7, minlength=NCT)       # messages per c-tile
    npad_ct = bpc * 128
    rows_padded = np.zeros((NCT, npad_ct), np.int64)
    clocal = np.full((NCT, npad_ct), -1.0, np.float32)
    offs = np.concatenate([[0], np.cumsum(cnt)])
    for t in range(NCT):
        n = cnt[t]
        rows_padded[t, :n] = r_s[offs[t] : offs[t + 1]]
        clocal[t, :n] = c_s[offs[t] : offs[t + 1]] & 127

    nblk = NCT * bpc
    rows_w = _wrap_idx(rows_padded.reshape(-1))                  # [128, NPAD//16]
    cloc = clocal.reshape(nblk, 128).T.copy()                    # [128, NBLK] f32
    dinv_t = dinv.reshape(NCT, 128).T.copy()                     # [128, 32]
    t2e_w = _wrap_idx(np.asarray(t2e))                           # [128, 256]
    scat = np.concatenate([np.asarray(e2t)[1:], [-1]])
    scat_w = _wrap_idx(scat)                                     # [128, 256]
    return rows_w, cloc, dinv_t, t2e_w, scat_w


KSTAGE = os.environ.get("KSTAGE", "full")  # debug bisect: gath|pphase|trans|full


def _build_program(bpc):
    """Build the SPMD Bass program (one core's view: SPC samples).

    All SWDGE gather/scatter instructions are chunked to <=1024 indices —
    a single instruction above the SWDGE descriptor-ring capacity hangs on
    hardware (2048 fails, 1024 passes)."""
    nblk = NCT * bpc
    npad = nblk * 128
    ch_blocks = 8                             # blocks per gather chunk (1024 idxs)
    ch_idx = ch_blocks * 128
    nch = (nblk + ch_blocks - 1) // ch_blocks # msg gather chunks per sample

    nc = bacc.Bacc("TRN2", target_bir_lowering=False, debug=False)

    t_emb_d = nc.dram_tensor("t_emb", [SPC, S, D], F32, kind="ExternalInput").ap()
    t2e_d = nc.dram_tensor("t2e_w", [SPC, 128, E // 16], I16, kind="ExternalInput").ap()
    rows_d = nc.dram_tensor("rows_w", [SPC, 128, npad // 16], I16, kind="ExternalInput").ap()
    cloc_d = nc.dram_tensor("cloc", [SPC, 128, nblk], F32, kind="ExternalInput").ap()
    scat_d = nc.dram_tensor("scat_w", [SPC, 128, E // 16], I16, kind="ExternalInput").ap()
    dinv_d = nc.dram_tensor("dinv_t", [SPC, 128, NCT], F32, kind="ExternalInput").ap()
    wt_d = nc.dram_tensor("wt", [KH, 128, D], BF16, kind="ExternalInput").ap()
    b_d = nc.dram_tensor("b_bc", [128, D], F32, kind="ExternalInput").ap()
    iota_d = nc.dram_tensor("iota_bf", [128, 128], BF16, kind="ExternalInput").ap()
    id_d = nc.dram_tensor("id_bf", [128, 128], BF16, kind="ExternalInput").ap()
    out_d = nc.dram_tensor("out", [SPC, S, D], F32, kind="ExternalOutput").ap()
    y0_d = nc.dram_tensor("y0_hbm", [SPC, E, D], BF16, kind="Internal").ap()

    with tile.TileContext(nc) as tc, ExitStack() as ctx:
        nc.gpsimd.load_library(library_config.mlp)

        cpool = ctx.enter_context(tc.tile_pool(name="const", bufs=1))
        meta = ctx.enter_context(tc.tile_pool(name="meta", bufs=2))
        epool = ctx.enter_context(tc.tile_pool(name="edge", bufs=1))
        ypool = ctx.enter_context(tc.tile_pool(name="y0", bufs=1))
        mpool = ctx.enter_context(tc.tile_pool(name="msg", bufs=2))
        ppool = ctx.enter_context(tc.tile_pool(name="pblk", bufs=4))
        zpool = ctx.enter_context(tc.tile_pool(name="z", bufs=1))
        ztpool = ctx.enter_context(tc.tile_pool(name="zt", bufs=1))
        capool = ctx.enter_context(tc.tile_pool(name="causal", bufs=2))
        ps_p = ctx.enter_context(tc.tile_pool(name="ps_p", bufs=3, space="PSUM"))
        ps_t = ctx.enter_context(tc.tile_pool(name="ps_t", bufs=2, space="PSUM"))
        ps_f = ctx.enter_context(tc.tile_pool(name="ps_f", bufs=2, space="PSUM"))

        # constants
        wt_sb = cpool.tile([128, KH, D], BF16)
        for kh in range(KH):
            nc.sync.dma_start(wt_sb[:, kh, :], wt_d[kh])
        b_sb = cpool.tile([128, D], F32)
        nc.sync.dma_start(b_sb[:], b_d[:])
        iota_sb = cpool.tile([128, 128], BF16)
        nc.sync.dma_start(iota_sb[:], iota_d[:])
        id_sb = cpool.tile([128, 128], BF16)
        nc.sync.dma_start(id_sb[:], id_d[:])

        deferred_scatters = []
        for s in range(SPC):
            # --- metadata loads
            t2e_sb = meta.tile([128, E // 16], I16, tag="t2e")
            nc.sync.dma_start(t2e_sb[:], t2e_d[s])
            rows_sb = meta.tile([128, npad // 16], I16, tag="rows")
            nc.sync.dma_start(rows_sb[:], rows_d[s])
            cloc_sb = meta.tile([128, nblk], F32, tag="cloc")
            nc.sync.dma_start(cloc_sb[:], cloc_d[s])
            scat_sb = meta.tile([128, E // 16], I16, tag="scat")
            nc.sync.dma_start(scat_sb[:], scat_d[s])
            dinv_sb = meta.tile([128, NCT], F32, tag="dinv")
            nc.sync.dma_start(dinv_sb[:], dinv_d[s])

            # --- edge gather: x = t_emb[t2e]  -> [128, 32, 256] f32
            edge_sb = epool.tile([128, NCT, D], F32)
            for c in range(E // 1024):
                nc.gpsimd.dma_gather(
                    edge_sb[:, c * 8 : (c + 1) * 8, :], t_emb_d[s],
                    t2e_sb[:, c * 64 : (c + 1) * 64], 1024, 1024, D,
                )

            # --- y0 = dinv * x -> bf16, then to HBM
            y0_sb = ypool.tile([128, NCT, D], BF16)
            for cb in range(NCT):
                nc.vector.tensor_scalar(
                    y0_sb[:, cb, :], edge_sb[:, cb, :],
                    dinv_sb[:, cb : cb + 1], None, op0=mybir.AluOpType.mult,
                )
            nc.sync.dma_start(
                y0_d[s].rearrange("(cb p) d -> p cb d", p=128), y0_sb[:]
            )

            # --- message gather chunks (8 blocks = 1024 idxs each) + P-phase
            z_sb = zpool.tile([128, NCT, D], BF16)
            if KSTAGE == "gath":
                # anchor gathers without P-phase: copy msgs into z cheaply
                for ch in range(nch):
                    t = mpool.tile([128, ch_blocks, D], BF16, tag="msg")
                    nc.gpsimd.dma_gather(
                        t[:], y0_d[s],
                        rows_sb[:, ch * (ch_idx // 16) : (ch + 1) * (ch_idx // 16)],
                        ch_idx, ch_idx, D)
                    if ch < NCT:
                        nc.vector.tensor_copy(z_sb[:, ch % NCT, :], t[:, 0, :])
                zf = capool.tile([128, NCT, D], F32, tag="zf")
                nc.vector.tensor_copy(zf[:], z_sb[:])
                nc.sync.dma_start(out_d[s][:E].rearrange("(cb p) d -> p cb d", p=128), zf[:])
                continue
            msg_tiles = {}

            def get_msg(ch):
                if ch not in msg_tiles:
                    t = mpool.tile([128, ch_blocks, D], BF16, tag="msg")
                    nc.gpsimd.dma_gather(
                        t[:], y0_d[s],
                        rows_sb[:, ch * (ch_idx // 16) : (ch + 1) * (ch_idx // 16)],
                        ch_idx, ch_idx, D,
                    )
                    msg_tiles[ch] = t
                return msg_tiles[ch]

            for ct in range(NCT):
                ps = ps_p.tile([128, D], F32, tag="agg")
                for j in range(bpc):
                    blk = ct * bpc + j
                    ch, sl = divmod(blk, ch_blocks)
                    msg_sb = get_msg(ch)
                    p_bf = ppool.tile([128, 128], BF16, tag="p")
                    nc.vector.tensor_scalar(
                        p_bf[:], iota_sb[:], cloc_sb[:, blk : blk + 1],
                        None, op0=mybir.AluOpType.is_equal,
                    )
                    nc.tensor.matmul(
                        ps[:], p_bf[:], msg_sb[:, sl, :],
                        start=(j == 0), stop=(j == bpc - 1),
                    )
                # z = dinv[c] * agg  (psum f32 -> sbuf bf16)
                nc.vector.tensor_scalar(
                    z_sb[:, ct, :], ps[:], dinv_sb[:, ct : ct + 1],
                    None, op0=mybir.AluOpType.mult,
                )

            if KSTAGE == "pphase":
                zf = capool.tile([128, NCT, D], F32, tag="zf")
                nc.vector.tensor_copy(zf[:], z_sb[:])
                nc.sync.dma_start(out_d[s][:E].rearrange("(cb p) d -> p cb d", p=128), zf[:])
                continue

            # --- transpose z -> z_T [2][128, 4096] bf16
            zt_sb = ztpool.tile([128, KH, E], BF16)
            for ct in range(NCT):
                for kh in range(KH):
                    pst = ps_t.tile([128, 128], BF16, tag="tr")
                    nc.tensor.transpose(
                        pst[:], z_sb[:, ct, kh * 128 : (kh + 1) * 128], id_sb[:]
                    )
                    nc.scalar.copy(
                        zt_sb[:, kh, ct * 128 : (ct + 1) * 128], pst[:]
                    )

            # --- gnn = z @ W.T + b  -> causal slots [128, 32, 256] f32
            causal_sb = capool.tile([128, NCT, D], F32)
            for ec in range(NCT):
                ps2 = ps_f.tile([128, D], F32, tag="mm")
                for kh in range(KH):
                    nc.tensor.matmul(
                        ps2[:],
                        zt_sb[:, kh, ec * 128 : (ec + 1) * 128],
                        wt_sb[:, kh, :],
                        start=(kh == 0), stop=(kh == KH - 1),
                    )
                nc.vector.tensor_tensor(
                    causal_sb[:, ec, :], ps2[:], b_sb[:], op=mybir.AluOpType.add
                )

            if KSTAGE == "trans":
                nc.sync.dma_start(out_d[s][:E].rearrange("(cb p) d -> p cb d", p=128), causal_sb[:])
                continue

            # --- output base copy; scatters deferred behind a barrier (the
            # copy+scatter+pipeline concurrency crashed the exec unit on HW)
            if KSTAGE != "nocopy":
                nc.sync.dma_start(out_d[s], t_emb_d[s])
            if KSTAGE == "noscat":
                continue
            deferred_scatters.append((s, causal_sb, scat_sb))

        # --- all scatter-adds after everything else has drained
        if deferred_scatters:
            tc.strict_bb_all_engine_barrier()
            for s, causal_sb, scat_sb in deferred_scatters:
                for c in range(E // 1024):
                    nreg = 1024 if c < E // 1024 - 1 else 1023  # trailing -1 pad
                    nc.gpsimd.dma_scatter_add(
                        out_d[s], causal_sb[:, c * 8 : (c + 1) * 8, :],
                        scat_sb[:, c * 64 : (c + 1) * 64], 1024, nreg, D,
                    )

    nc.compile()
    return nc


def kernel(token_embeddings, tokens2edges, edge_index, edges2tokens, W, b):
    token_embeddings = np.ascontiguousarray(np.asarray(token_embeddings, dtype=np.float32))
    tokens2edges = np.asarray(tokens2edges)
    edge_index = np.asarray(edge_index)
    edges2tokens = np.asarray(edges2tokens)
    W = np.asarray(W, dtype=np.float32)
    b = np.asarray(b, dtype=np.float32)

    # global uniform blocks-per-ctile so all cores share one program
    bpc = 0
    for bi in range(B):
        col = edge_index[bi, 1].astype(np.int64)
        cnt = np.bincount(
            np.concatenate([col, np.arange(E)]) >> 7, minlength=NCT
        )
        bpc = max(bpc, int(np.max((cnt + 127) // 128)))

    preps = [
        _prep_sample(
            edge_index[bi, 0].astype(np.int64),
            edge_index[bi, 1].astype(np.int64),
            tokens2edges[bi], edges2tokens[bi], bpc,
        )
        for bi in range(B)
    ]

    wt_host = np.ascontiguousarray(W.T).astype(BF16_NP).reshape(KH, 128, D)
    b_bc = np.tile(b[None, :], (128, 1)).astype(np.float32)
    iota_bf = np.tile(np.arange(128, dtype=np.float32)[None, :], (128, 1)).astype(BF16_NP)
    id_bf = np.eye(128, dtype=np.float32).astype(BF16_NP)

    nc = _build_program(bpc)

    in_maps = []
    for c in range(NCORES):
        sl = slice(c * SPC, (c + 1) * SPC)
        rows_w = np.stack([preps[bi][0] for bi in range(sl.start, sl.stop)])
        cloc = np.stack([preps[bi][1] for bi in range(sl.start, sl.stop)])
        dinv_t = np.stack([preps[bi][2] for bi in range(sl.start, sl.stop)])
        t2e_w = np.stack([preps[bi][3] for bi in range(sl.start, sl.stop)])
        scat_w = np.stack([preps[bi][4] for bi in range(sl.start, sl.stop)])
        in_maps.append({
            "t_emb": np.ascontiguousarray(token_embeddings[sl]),
            "t2e_w": t2e_w, "rows_w": rows_w, "cloc": cloc,
            "scat_w": scat_w, "dinv_t": dinv_t,
            "wt": wt_host, "b_bc": b_bc, "iota_bf": iota_bf, "id_bf": id_bf,
        })

    res = run_bass_kernel_spmd(nc, in_maps, list(range(NCORES)))
    out = np.concatenate([r["out"] for r in res.results], axis=0)
    return out.astype(np.float32)
